# revision 1
# baseline (speedup 1.0000x reference)
"""Trainium2 Bass kernel for nn_KATLayer (KAT basis-function layer).

out[b,o] = sum_{i,n} exp(-z^2) * (1 + erf(alpha*z/sqrt(2))) * w[i,o,n]
  z = (x[b,i] - c[i,o,n]) / (|sigma|+1e-8),  c = |scale|*mx_start + mx_train

Sharding: output dim O split across 8 cores (O_shard=64). Per core:
  partitions = i (4 chunks of 128), free = (o_local, n) = 1024 per tile,
  one tile per (b, i_chunk) = 128 tiles.

Per tile:
  DVE:  zm = (c - x)*rinv          [scalar_tensor_tensor, x per-partition]
        um = (c - x)*A             [A = alpha*rinv/sqrt(2)]
  ACT:  e  = Derivative_Erf(zm)    [= 2/sqrt(pi)*exp(-z^2), even in z]
        t  = Erf(-um)              [= erf(alpha*z/sqrt(2))]
  DVE:  s  = (t + 1)*wt            [wt = w*sqrt(pi)/2]
        p  = e*s
  PE :  psum[b, half] += onehot_b.T @ p_half   (reduces over i-partitions)
Final: DVE reduce over n: psum(32,64,16) -> (32,64); DMA out.

Derivative_Erf and Erf live in different ACT table sets (~2.7us/switch), so
tiles are processed in groups with phase-batched activations (2 switches per
group). Phase order within a group keeps zm and um lifetimes disjoint so
both phases share one SBUF pool.

fp16 variant (default): e/t/s/p and the PE-reduce stream run in fp16
(DVE 2x mode, PE 1 cyc/row); zm/um stay fp32 (z precision is critical).
"""
import sys

sys.path.insert(0, "/opt/trn_rl_repo")
import math

import numpy as np

B, I, O, N = 32, 512, 512, 16
NCORES = 8
OS = O // NCORES          # 64 output dims per core
KC = I // 128             # 4 i-chunks
P = 128
G = 12                    # tiles per activation-phase group
INV_SQRT2 = 0.7071067811865476
SQRT_PI_2 = math.sqrt(math.pi) / 2.0

_CACHE = {}
LAST_RESULTS = None


def _build_nc(reps=1, fp16_products=True, G=G):
    import concourse.bacc as bacc
    import concourse.mybir as mybir
    from concourse import tile

    fp32 = mybir.dt.float32
    fp16 = mybir.dt.float16
    pdt = fp16 if fp16_products else fp32
    AF = mybir.ActivationFunctionType
    ALU = mybir.AluOpType

    nc = bacc.Bacc(
        "TRN2", target_bir_lowering=False, debug=False, num_devices=NCORES
    )
    c_d = nc.dram_tensor("c", [KC, P, OS, N], fp32, kind="ExternalInput")
    r_d = nc.dram_tensor("r", [KC, P, OS, N], fp32, kind="ExternalInput")
    a_d = nc.dram_tensor("a", [KC, P, OS, N], fp32, kind="ExternalInput")
    w_d = nc.dram_tensor("w", [KC, P, OS, N], pdt, kind="ExternalInput")
    x_d = nc.dram_tensor("x", [KC, P, B], fp32, kind="ExternalInput")
    oh_d = nc.dram_tensor("oh", [P, B, B], pdt, kind="ExternalInput")
    out_d = nc.dram_tensor("out", [B, OS], fp32, kind="ExternalOutput")

    with tile.TileContext(nc) as tc:
        with (
            tc.tile_pool(name="const", bufs=1) as cpool,
            tc.tile_pool(name="zu", bufs=G + 2) as zup,
            tc.tile_pool(name="et", bufs=2 * G + 3) as etp,
            tc.tile_pool(name="sp", bufs=3) as spool,
            tc.tile_pool(name="pp", bufs=3) as ppool,
            tc.tile_pool(name="psum", bufs=1, space="PSUM") as psp,
            tc.tile_pool(name="outp", bufs=1) as opool,
        ):
            c_sb, r_sb, a_sb, w_sb = [], [], [], []
            for k in range(KC):
                for lst, dram, nm, dt_ in (
                    (c_sb, c_d, "c", fp32),
                    (r_sb, r_d, "r", fp32),
                    (a_sb, a_d, "a", fp32),
                    (w_sb, w_d, "w", pdt),
                ):
                    t = cpool.tile([P, OS, N], dt_, tag=f"{nm}{k}")
                    nc.sync.dma_start(t[:], dram[k])
                    lst.append(t)
            x_sb = cpool.tile([P, KC * B], fp32, tag="x")
            for k in range(KC):
                nc.sync.dma_start(x_sb[:, k * B : (k + 1) * B], x_d[k])
            oh_sb = cpool.tile([P, B, B], pdt, tag="oh")
            nc.sync.dma_start(oh_sb[:], oh_d[:])

            psum_t = psp.tile([B, OS, N], fp32)
            tiles = [(b, k) for b in range(B) for k in range(KC)]
            out_sb = opool.tile([B, OS], fp32)

            for rep in range(reps):
                n_mm = 0
                for g0 in range(0, len(tiles), G):
                    grp = tiles[g0 : g0 + G]
                    # phase 1a: all zm; phase 1b: DerivErf(zm) -> e (fp16)
                    es, ts_ = [], []
                    for b, k in grp:
                        xcol = x_sb[:, k * B + b : k * B + b + 1]
                        zm = zup.tile([P, OS, N], fp32, tag="zu")
                        nc.vector.scalar_tensor_tensor(
                            zm[:], c_sb[k][:], xcol, r_sb[k][:],
                            op0=ALU.subtract, op1=ALU.mult,
                        )
                        e = etp.tile([P, OS, N], pdt, tag="et")
                        es.append((zm, e))
                    for zm, e in es:
                        nc.scalar.activation(e[:], zm[:], AF.Derivative_Erf)
                    # phase 2a: all um; phase 2b: Erf(-um) -> t (fp16)
                    for b, k in grp:
                        xcol = x_sb[:, k * B + b : k * B + b + 1]
                        um = zup.tile([P, OS, N], fp32, tag="zu")
                        nc.vector.scalar_tensor_tensor(
                            um[:], c_sb[k][:], xcol, a_sb[k][:],
                            op0=ALU.subtract, op1=ALU.mult,
                        )
                        t_ = etp.tile([P, OS, N], pdt, tag="et")
                        ts_.append((um, t_))
                    for um, t_ in ts_:
                        nc.scalar.activation(t_[:], um[:], AF.Erf, scale=-1.0)
                    # phase 3: s = (t+1)*wt ; p = e*s ; PE-reduce over i
                    for (b, k), (zm, e), (um, t_) in zip(grp, es, ts_):
                        s = spool.tile([P, OS, N], pdt)
                        nc.vector.scalar_tensor_tensor(
                            s[:], t_[:], 1.0, w_sb[k][:],
                            op0=ALU.add, op1=ALU.mult,
                        )
                        p = ppool.tile([P, OS, N], pdt)
                        nc.vector.tensor_mul(p[:], e[:], s[:])
                        for h in range(2):
                            nc.tensor.matmul(
                                psum_t[:, 32 * h : 32 * (h + 1), :],
                                oh_sb[:, b, :],
                                p[:, 32 * h : 32 * (h + 1), :],
                                start=(n_mm < 2),
                                stop=(n_mm >= 2 * len(tiles) - 2),
                            )
                            n_mm += 1

                nc.vector.tensor_reduce(
                    out_sb[:], psum_t[:], axis=mybir.AxisListType.X, op=ALU.add
                )
            nc.sync.dma_start(out_d[:], out_sb[:])

    nc.compile()
    return nc


def _prep_inputs(x, mx_train, scale, sigma, alpha, w, mx_start,
                 fp16_products=True):
    pdt = np.float16 if fp16_products else np.float32
    c = (np.abs(scale)[:, :, None] * mx_start[None, None, :]
         + mx_train[:, :, None]).astype(np.float32)
    rinv = (1.0 / (np.abs(sigma) + 1e-8)).astype(np.float32)
    A = (alpha * INV_SQRT2 * rinv).astype(np.float32)
    wt = (w * SQRT_PI_2).astype(pdt)
    xT = np.ascontiguousarray(x.T.reshape(KC, P, B)).astype(np.float32)
    oh = np.broadcast_to(np.eye(B, dtype=pdt), (P, B, B))
    oh = np.ascontiguousarray(oh)

    in_maps = []
    for d in range(NCORES):
        sl = slice(d * OS, (d + 1) * OS)
        in_maps.append({
            "c": np.ascontiguousarray(c[:, sl].reshape(KC, P, OS, N)),
            "r": np.ascontiguousarray(rinv[:, sl].reshape(KC, P, OS, N)),
            "a": np.ascontiguousarray(A[:, sl].reshape(KC, P, OS, N)),
            "w": np.ascontiguousarray(wt[:, sl].reshape(KC, P, OS, N)),
            "x": xT,
            "oh": oh,
        })
    return in_maps


def kernel(x, mx_train, scale, sigma, alpha, w, mx_start, _trace=False):
    global LAST_RESULTS
    from concourse.bass_utils import run_bass_kernel_spmd

    if "nc" not in _CACHE:
        _CACHE["nc"] = _build_nc()
    nc = _CACHE["nc"]
    in_maps = _prep_inputs(
        np.asarray(x, np.float32), np.asarray(mx_train, np.float32),
        np.asarray(scale, np.float32), np.asarray(sigma, np.float32),
        np.asarray(alpha, np.float32), np.asarray(w, np.float32),
        np.asarray(mx_start, np.float32),
    )
    res = run_bass_kernel_spmd(nc, in_maps, core_ids=list(range(NCORES)),
                               trace=_trace)
    LAST_RESULTS = res
    return np.concatenate([r["out"] for r in res.results], axis=1)



# revision 2
# speedup vs baseline: 1.1851x; 1.1851x over previous
"""Trainium2 Bass kernel for nn_KATLayer (KAT basis-function layer).

out[b,o] = sum_{i,n} exp(-z^2) * (1 + erf(alpha*z/sqrt(2))) * w[i,o,n]
  z = (x[b,i] - c[i,o,n]) / (|sigma|+1e-8),  c = |scale|*mx_start + mx_train

Sharding: output dim O split across 8 cores (OS=64 per core). Per core the
16.7M basis evaluations are tiled as partitions=i (KC=4 chunks of 128),
free=(o_local,n)=1024, one logical tile per (k, b); tiles are processed in
groups of B2=4 consecutive b sharing one k so the per-(k)-constant tensors
(re/ae/wt) broadcast across the group dim with stride-0 APs and the fp16
DVE ops run at 2x/4x rate on 4096-wide instructions.

Factorization (host precomputes): y = (c - x)*48 [fp32->fp16], then
  zm = y*re     re = rinv/48            (TT fp16 2x)   -> e = DerivErf(zm)
  um = y*ae     ae = -alpha*re/sqrt(2)  (TT fp16 2x)   -> t = Erf(um)
  t1 = t+1  (TS fp16 imm,imm 4x, in-place)
  q  = e*t1 (TT 2x, in-place on e);  p = q*wt (TT 2x, in-place)
  psum[b,(o,n)] += onehot_b.T @ p    (PE reduces i)
  out = reduce_n(psum)               (DVE, once)

y runs on GPSIMD (plain TT subtract, c48 - x48 broadcast) to off-load the
DVE, which is the bottleneck engine. Activations are phase-batched PG
groups at a time (all DerivErf, then all Erf) to amortize the ACT table
switch (~1.3us); DVE tail work of superstep s-1 is emitted after the head
work of superstep s so the in-order DVE queue never stalls on ACT.
"""
import sys

sys.path.insert(0, "/opt/trn_rl_repo")
import math

import numpy as np

B, I, O, N = 32, 512, 512, 16
NCORES = 8
OS = O // NCORES          # 64 output dims per core
KC = I // 128             # 4 i-chunks
P = 128
FREE = OS * N             # 1024
B2 = 4                    # b-tiles fused per instruction group
PG = 3                    # groups per activation phase batch
RSCALE = 48.0             # |y| <= ~65, fp16-safe; re = rinv/RSCALE ~ O(1)
INV_SQRT2 = 0.7071067811865476
SQRT_PI_2 = math.sqrt(math.pi) / 2.0

Y_ENGINE = "gpsimd"       # "gpsimd" | "dve"
P_GPSIMD_EVERY = 0        # 0 = p always on DVE; k>0 = every k-th group on gpsimd

_CACHE = {}
LAST_RESULTS = None


def _build_nc():
    import concourse.bacc as bacc
    import concourse.mybir as mybir
    from concourse import tile

    fp32 = mybir.dt.float32
    fp16 = mybir.dt.float16
    AF = mybir.ActivationFunctionType
    ALU = mybir.AluOpType

    nc = bacc.Bacc(
        "TRN2", target_bir_lowering=False, debug=False, num_devices=NCORES
    )
    c_d = nc.dram_tensor("c", [KC, P, FREE], fp32, kind="ExternalInput")
    re_d = nc.dram_tensor("re", [KC, P, FREE], fp16, kind="ExternalInput")
    ae_d = nc.dram_tensor("ae", [KC, P, FREE], fp16, kind="ExternalInput")
    wt_d = nc.dram_tensor("wt", [KC, P, FREE], fp16, kind="ExternalInput")
    x_d = nc.dram_tensor("x", [KC, P, B], fp32, kind="ExternalInput")
    oh_d = nc.dram_tensor("oh", [P, B, B], fp16, kind="ExternalInput")
    out_d = nc.dram_tensor("out", [B, OS], fp32, kind="ExternalOutput")

    # tile list: groups of B2 consecutive b sharing one k
    groups = [(k, 4 * bg) for k in range(KC) for bg in range(B // B2)]
    n_tiles = KC * B

    with tile.TileContext(nc) as tc:
        with (
            tc.tile_pool(name="const", bufs=1) as cp,
            tc.tile_pool(name="yp", bufs=3) as yp,
            tc.tile_pool(name="zmp", bufs=PG + 1) as zmp,
            tc.tile_pool(name="ump", bufs=PG + 1) as ump,
            tc.tile_pool(name="ep", bufs=PG + 1) as ep,
            tc.tile_pool(name="tp", bufs=PG) as tp,
            tc.tile_pool(name="psum", bufs=1, space="PSUM") as psp,
            tc.tile_pool(name="outp", bufs=1) as op_,
        ):
            c_sb, re_sb, ae_sb, wt_sb = [], [], [], []
            for k in range(KC):
                for lst, dram, nm, dt_ in (
                    (c_sb, c_d, "c", fp32),
                    (re_sb, re_d, "re", fp16),
                    (ae_sb, ae_d, "ae", fp16),
                    (wt_sb, wt_d, "wt", fp16),
                ):
                    t = cp.tile([P, FREE], dt_, tag=f"{nm}{k}")
                    nc.sync.dma_start(t[:], dram[k])
                    lst.append(t)
            x_sb = cp.tile([P, KC * B], fp32, tag="x")
            for k in range(KC):
                nc.sync.dma_start(x_sb[:, k * B : (k + 1) * B], x_d[k])
            oh_sb = cp.tile([P, B, B], fp16, tag="oh")
            nc.sync.dma_start(oh_sb[:], oh_d[:])

            psum_t = psp.tile([B, OS, N], fp32)
            out_sb = op_.tile([B, OS], fp32)

            state = {"n_mm": 0}

            def head(g):
                """y + zm/um + ACT-e for group g (Erf act deferred)."""
                k, b0 = groups[g]
                y = yp.tile([P, B2, FREE], fp16, tag="y")
                for j in range(B2):
                    xcol = x_sb[:, k * B + b0 + j : k * B + b0 + j + 1]
                    if Y_ENGINE == "gpsimd":
                        nc.gpsimd.tensor_tensor(
                            y[:, j, :], c_sb[k][:],
                            xcol.broadcast_to([P, FREE]),
                            op=ALU.subtract,
                        )
                    else:
                        nc.vector.tensor_scalar(
                            y[:, j, :], c_sb[k][:], xcol, None,
                            op0=ALU.subtract,
                        )
                re_b = re_sb[k][:].unsqueeze(1).broadcast_to([P, B2, FREE])
                ae_b = ae_sb[k][:].unsqueeze(1).broadcast_to([P, B2, FREE])
                zm = zmp.tile([P, B2, FREE], fp16, tag="zm")
                um = ump.tile([P, B2, FREE], fp16, tag="um")
                nc.vector.tensor_tensor(zm[:], y[:], re_b, op=ALU.mult)
                nc.vector.tensor_tensor(um[:], y[:], ae_b, op=ALU.mult)
                e = ep.tile([P, B2, FREE], fp16, tag="e")
                nc.scalar.activation(e[:], zm[:], AF.Derivative_Erf)
                return um, e

            def erf_phase(um, t_tile):
                nc.scalar.activation(t_tile[:], um[:], AF.Erf)

            def tail(g, e, t_tile):
                """t1/q/p on DVE (+PE matmuls) for group g."""
                k, b0 = groups[g]
                wt_b = wt_sb[k][:].unsqueeze(1).broadcast_to([P, B2, FREE])
                nc.vector.tensor_scalar(
                    t_tile[:], t_tile[:], 1.0, 1.0, op0=ALU.add, op1=ALU.mult
                )
                nc.vector.tensor_tensor(e[:], e[:], t_tile[:], op=ALU.mult)
                use_gp = P_GPSIMD_EVERY and (g % P_GPSIMD_EVERY == 0)
                eng = nc.gpsimd if use_gp else nc.vector
                eng.tensor_tensor(e[:], e[:], wt_b, op=ALU.mult)
                for j in range(B2):
                    b = b0 + j
                    for h in range(2):
                        nc.tensor.matmul(
                            psum_t[:, 32 * h : 32 * (h + 1), :],
                            oh_sb[:, b, :],
                            e[:, j, 512 * h : 512 * (h + 1)],
                            start=(state["n_mm"] < 2),
                            stop=(state["n_mm"] >= 2 * n_tiles - 2),
                        )
                        state["n_mm"] += 1

            # software-pipelined supersteps of PG groups
            pending = []  # list of (g, e, t_tile) with Erf issued, tail not
            for s0 in range(0, len(groups), PG):
                ss = range(s0, min(s0 + PG, len(groups)))
                heads = [(g, *head(g)) for g in ss]
                for g, um, e in heads:
                    t_tile = tp.tile([P, B2, FREE], fp16, tag="t")
                    erf_phase(um, t_tile)
                    pending.append((g, e, t_tile))
                # emit tails for the PREVIOUS superstep (ACT has finished them
                # by the time DVE drains this superstep's head work)
                while len(pending) > PG:
                    tail(*pending.pop(0))
            while pending:
                tail(*pending.pop(0))

            nc.vector.tensor_reduce(
                out_sb[:], psum_t[:], axis=mybir.AxisListType.X, op=ALU.add
            )
            nc.sync.dma_start(out_d[:], out_sb[:])

    nc.compile()
    return nc


def _prep_inputs(x, mx_train, scale, sigma, alpha, w, mx_start):
    c = (np.abs(scale)[:, :, None] * mx_start[None, None, :]
         + mx_train[:, :, None]).astype(np.float32)
    rinv = (1.0 / (np.abs(sigma) + 1e-8)).astype(np.float32)
    re = (rinv / RSCALE).astype(np.float16)
    ae = (-alpha * (rinv / RSCALE) * INV_SQRT2).astype(np.float16)
    wt = (w * SQRT_PI_2).astype(np.float16)
    if Y_ENGINE == "gpsimd":
        c_in = (c * RSCALE).astype(np.float32)
        x_in = (x * RSCALE).astype(np.float32)
    else:
        c_in = c
        x_in = x.astype(np.float32)
        # dve path computes y=(c-x) unscaled; fold RSCALE into re/ae instead
        re = (rinv).astype(np.float16)
        ae = (-alpha * rinv * INV_SQRT2).astype(np.float16)
    xT = np.ascontiguousarray(x_in.T.reshape(KC, P, B)).astype(np.float32)
    oh = np.ascontiguousarray(
        np.broadcast_to(np.eye(B, dtype=np.float16), (P, B, B)))

    in_maps = []
    for d in range(NCORES):
        sl = slice(d * OS, (d + 1) * OS)
        in_maps.append({
            "c": np.ascontiguousarray(c_in[:, sl].reshape(KC, P, FREE)),
            "re": np.ascontiguousarray(re[:, sl].reshape(KC, P, FREE)),
            "ae": np.ascontiguousarray(ae[:, sl].reshape(KC, P, FREE)),
            "wt": np.ascontiguousarray(wt[:, sl].reshape(KC, P, FREE)),
            "x": xT,
            "oh": oh,
        })
    return in_maps


def kernel(x, mx_train, scale, sigma, alpha, w, mx_start, _trace=False):
    global LAST_RESULTS
    from concourse.bass_utils import run_bass_kernel_spmd

    if "nc" not in _CACHE:
        _CACHE["nc"] = _build_nc()
    nc = _CACHE["nc"]
    in_maps = _prep_inputs(
        np.asarray(x, np.float32), np.asarray(mx_train, np.float32),
        np.asarray(scale, np.float32), np.asarray(sigma, np.float32),
        np.asarray(alpha, np.float32), np.asarray(w, np.float32),
        np.asarray(mx_start, np.float32),
    )
    res = run_bass_kernel_spmd(nc, in_maps, core_ids=list(range(NCORES)),
                               trace=_trace)
    LAST_RESULTS = res
    return np.concatenate([r["out"] for r in res.results], axis=1)
